# revision 12
# baseline (speedup 1.0000x reference)
"""Dentate gyrus circuit simulation kernel for 8 Trainium2 NeuronCores.

Strategy
--------
Shard the 20000 GC neurons across 8 cores (2500 each, padded to 2560 =
128 partitions x 20 columns). All weights live SBUF-resident in fp16;
every per-step matvec runs on the TensorEngine with the weight tile as
the stationary operand (lhsT) and the spike vector column as the moving
operand, so results land partition-parallel in PSUM.

The MEC spike train is a known input, so the mec->gc and mec->pv
contributions for all T steps are precomputed with one real matmul at
kernel start, removing W_mec_* from the per-step loop entirely.

Per step, each core:
  - computes local GC spikes + (replicated) small-pop spikes,
  - GC-outgoing partial increments to [mc|pv|sst] targets (sum over its
    2500 GC) plus its 256-column shard of the small->small increments,
  - publishes the [128, 18] fp32 partial via AllGather (8 cores),
  - computes GC-incoming increments from the replicated small spikes,
  - updates GC state locally and small-pop state redundantly
    (identically on every core) from the gathered partials.

Small-population canonical layout (partition-minor, id = p + 128*f):
  mc at [0,600) cols 0:5, pv at [640,1240) cols 5:10,
  sst at [1280,1680) cols 10:14; 1792 = 14 cols total, pads have zero
  weights. GC per-core canonical id c = p + 128*f, cols 0:20.
"""
import sys
sys.path.insert(0, '/opt/trn_rl_repo')
import numpy as np

import concourse.bass as bass
import concourse.mybir as mybir
import concourse.tile as tile
from concourse.bass_utils import run_bass_kernel_spmd

F32 = mybir.dt.float32
WDT = mybir.dt.float16
AF = mybir.ActivationFunctionType

N_GC, N_MC, N_PV, N_SST, N_MEC = 20000, 600, 600, 400, 1200
T = 100
DT = 0.1
TAU_AMPA, TAU_GABA, TAU_M = 2.0, 10.0, 20.0
V_REST, V_TH, V_RESET = -70.0, -50.0, -70.0
D_AMPA = float(np.exp(-DT / TAU_AMPA))
D_GABA = float(np.exp(-DT / TAU_GABA))

NCORES = 8
G = N_GC // NCORES          # 2500 gc per core
GM = 20                     # gc column count (2560 = 128*20)
GP = 128 * GM
SM = 14                     # small-pop column count (1792 = 128*14)
SMP = 128 * SM
SSB = 2                     # small->small target cols per core (256 ids)
# k-tile counts
KT_MC, KT_PV, KT_SST = 5, 5, 4
KT_INI = KT_PV + KT_SST     # incoming inhibitory k-tiles
KT_MEC = 10                 # mec padded to 1280
PAY = SM + 2 * SSB          # collective payload cols: po(14)|ssE(2)|ssI(2)


class SafeTileContext(tile.TileContext):
    """TileContext whose exit drain carries no sem waits (this walrus
    rejects sync waits on the Drain encoding); waits go on single-wait
    NOPs instead."""

    def _drain_and_barrier(self, tick_clock, wait_clock):
        probe = self.nc.sync.nop(nofuse=True, hint="exit_wait_probe")
        wait_clock.add_sem_waits(
            probe.ins, tile.ScopedClock({None: tick_clock.global_clock})
        )
        waits = list(probe.ins.sync_info.on_wait) if probe.ins.sync_info else []
        if probe.ins.sync_info is not None:
            probe.ins.sync_info = None
        for w in waits:
            n = self.nc.sync.nop(nofuse=True, hint="exit_wait")
            n.ins.sync_info = mybir.SyncInfo(on_wait=[w], on_update=[])
        self.nc.sync.drain()
        self.nc.all_engine_barrier()
        assert self.sems is not None
        popped = self.nc._tile_sem_poison_stack.pop()
        assert popped is self._sem_poison
        self.nc.clear_and_free_semaphores(list(self.sems.allocated().values()))
        self.nc.all_engine_barrier()


def _split_waits(nc, k=1):
    """Hoist sem waits beyond ``k`` per instruction onto fresh same-engine
    NOPs placed immediately before — this walrus build rejects multi-wait
    sync commands on several instruction encodings."""
    ctr = 0
    for bb in nc.main_func.blocks:
        insts = list(bb.instructions)
        out = []
        changed = False
        for inst in insts:
            si = inst.sync_info
            if si is not None and si.on_wait and len(si.on_wait) > k:
                waits = list(si.on_wait)
                extra, keep = waits[:-k], waits[-k:]
                for i in range(0, len(extra), k):
                    nop = mybir.InstNoOp(name=f"waitnop-{ctr}", ins=[], outs=[])
                    ctr += 1
                    nop.engine = inst.engine
                    nop.sync_info = mybir.SyncInfo(
                        on_wait=extra[i:i + k], on_update=[])
                    nc.register_instruction(nop)
                    out.append(nop)
                inst.sync_info = mybir.SyncInfo(
                    on_wait=keep, on_update=list(si.on_update))
                changed = True
            out.append(inst)
        if changed:
            bb.instructions[:] = out
    return ctr


def _upd(nc, pool, V, gE, gI, psE, psI, s_f32, ncols, drive_e=None):
    """One Euler step + soft reset for a population laid out [128, ncols].

    gE/gI decay + increment, I = gE*(0-V) + gI*(-70-V),
    Vn = V + DT*((V_REST-V)/TAU_M + I), V' = Vn - (Vn - V_RESET)*s.
    """
    a = 1.0 - DT / TAU_M
    b = (DT / TAU_M) * V_REST
    t1 = pool.tile([128, ncols], F32, name=f"t1_{ncols}", tag=f"t1_{ncols}")
    t2 = pool.tile([128, ncols], F32, name=f"t2_{ncols}", tag=f"t2_{ncols}")
    t3 = pool.tile([128, ncols], F32, name=f"t3_{ncols}", tag=f"t3_{ncols}")
    nc.vector.tensor_scalar_mul(gE[:], gE[:], D_AMPA)
    nc.vector.tensor_add(gE[:], gE[:], psE)
    if drive_e is not None:
        dst, src = drive_e
        nc.vector.tensor_add(dst, dst, src)
    nc.vector.tensor_scalar_mul(gI[:], gI[:], D_GABA)
    nc.vector.tensor_add(gI[:], gI[:], psI)
    nc.vector.tensor_add(t1[:], gE[:], gI[:])          # w = gE + gI
    nc.vector.tensor_mul(t1[:], t1[:], V[:])           # u = w * V
    nc.vector.tensor_scalar(t2[:], V[:], a, b, mybir.AluOpType.mult,
                            mybir.AluOpType.add)       # Vn = a*V + b
    nc.vector.tensor_scalar_mul(t1[:], t1[:], -DT)
    nc.vector.tensor_add(t2[:], t2[:], t1[:])          # Vn -= DT*u
    nc.vector.tensor_scalar_mul(t3[:], gI[:], -70.0 * DT)
    nc.vector.tensor_add(t2[:], t2[:], t3[:])          # Vn -= 70*DT*gI
    # soft reset: V' = Vn - (Vn - V_RESET)*s
    nc.vector.tensor_scalar_add(t3[:], t2[:], -V_RESET)
    nc.vector.tensor_mul(t3[:], t3[:], s_f32[:])
    nc.vector.tensor_sub(V[:], t2[:], t3[:])


def build_program(t_steps):
    nc = bass.Bass(num_devices=NCORES)

    w_ine_d = nc.dram_tensor("w_ine", [128, KT_MC * GP], WDT, kind="ExternalInput")
    w_ini_d = nc.dram_tensor("w_ini", [128, KT_INI * GP], WDT, kind="ExternalInput")
    w_out_d = nc.dram_tensor("w_out", [128, GM * SMP], WDT, kind="ExternalInput")
    w_ss_d = nc.dram_tensor("w_ss", [128, SM * SSB * 128], WDT, kind="ExternalInput")
    w_mecgc_d = nc.dram_tensor("w_mecgc", [128, KT_MEC * GP], WDT, kind="ExternalInput")
    w_mecpv_d = nc.dram_tensor("w_mecpv", [128, KT_MEC * KT_PV * 128], WDT, kind="ExternalInput")
    mect_d = nc.dram_tensor("mect", [128, KT_MEC * t_steps], WDT, kind="ExternalInput")
    vg_out_d = nc.dram_tensor("vg_out", [t_steps, 128, GM], F32, kind="ExternalOutput")

    rg = [list(range(NCORES))]

    with SafeTileContext(nc) as tc:
        with tc.tile_pool(name="persist", bufs=1) as wpool:
            drive_gc = wpool.tile([128, GM * t_steps], F32)   # col m*T + t
            drive_pv = wpool.tile([128, KT_PV * t_steps], F32)

            Vg = wpool.tile([128, GM], F32)
            gEg = wpool.tile([128, GM], F32)
            gIg = wpool.tile([128, GM], F32)
            Vs = wpool.tile([128, SM], F32)
            gEs = wpool.tile([128, SM], F32)
            gIs = wpool.tile([128, SM], F32)
            incE = wpool.tile([128, SM], F32)
            incI = wpool.tile([128, SM], F32)
            bias_th = wpool.tile([128, 1], F32)
            nc.vector.memset(bias_th[:], -V_TH / 2.0)
            nc.vector.memset(Vg[:], V_REST)
            nc.vector.memset(gEg[:], 0.0)
            nc.vector.memset(gIg[:], 0.0)
            nc.vector.memset(Vs[:], V_REST)
            nc.vector.memset(gEs[:], 0.0)
            nc.vector.memset(gIs[:], 0.0)

            # ---- Phase 1: precompute mec drives, then free mec weights ----
            with (
                tc.tile_pool(name="mecpool", bufs=1) as mecpool,
                tc.tile_pool(name="pcpsum", bufs=4, space="PSUM") as pcpsum,
            ):
                wm_gc = mecpool.tile([128, KT_MEC * GP], WDT)
                wm_pv = mecpool.tile([128, KT_MEC * KT_PV * 128], WDT)
                smect = mecpool.tile([128, KT_MEC * t_steps], WDT)
                nc.sync.dma_start(wm_gc[:], w_mecgc_d[:])
                nc.sync.dma_start(wm_pv[:], w_mecpv_d[:])
                nc.sync.dma_start(smect[:], mect_d[:])
                for m in range(GM):
                    ps = pcpsum.tile([128, t_steps], F32, name="pcps", tag="pcps")
                    for k in range(KT_MEC):
                        nc.tensor.matmul(
                            ps[:], wm_gc[:, (k * GM + m) * 128:(k * GM + m + 1) * 128],
                            smect[:, k * t_steps:(k + 1) * t_steps],
                            start=(k == 0), stop=(k == KT_MEC - 1))
                    nc.vector.tensor_copy(drive_gc[:, m * t_steps:(m + 1) * t_steps], ps[:])
                for m in range(KT_PV):
                    ps = pcpsum.tile([128, t_steps], F32, name="pcps", tag="pcps")
                    for k in range(KT_MEC):
                        nc.tensor.matmul(
                            ps[:], wm_pv[:, (k * KT_PV + m) * 128:(k * KT_PV + m + 1) * 128],
                            smect[:, k * t_steps:(k + 1) * t_steps],
                            start=(k == 0), stop=(k == KT_MEC - 1))
                    nc.vector.tensor_copy(drive_pv[:, m * t_steps:(m + 1) * t_steps], ps[:])

            # ---- Phase 2: resident weights (reuse freed mec space) ----
            with (
                tc.tile_pool(name="mainw", bufs=1) as mainw,
                tc.tile_pool(name="sppool", bufs=3) as sppool,
                tc.tile_pool(name="tmppool", bufs=2) as tmppool,
                tc.tile_pool(name="agpool", bufs=2) as agpool,
                tc.tile_pool(name="gcpsum", bufs=2, space="PSUM") as gcpsum,
                tc.tile_pool(name="smpsum", bufs=2, space="PSUM") as smpsum,
                tc.tile_pool(name="dram", bufs=2, space="DRAM") as dram,
            ):
                w_ine = mainw.tile([128, KT_MC * GP], WDT)
                w_ini = mainw.tile([128, KT_INI * GP], WDT)
                w_out = mainw.tile([128, GM * SMP], WDT)
                w_ss = mainw.tile([128, SM * SSB * 128], WDT)
                nc.sync.dma_start(w_ine[:], w_ine_d[:])
                nc.sync.dma_start(w_ini[:], w_ini_d[:])
                nc.sync.dma_start(w_out[:], w_out_d[:])
                nc.sync.dma_start(w_ss[:], w_ss_d[:])

                for t in range(t_steps):
                    # spikes: s = sigmoid((V - V_TH)/2) = sigmoid(0.5*V + 10)
                    sgf = sppool.tile([128, GM], F32, name="sgf", tag="sgf")
                    ssf = sppool.tile([128, SM], F32, name="ssf", tag="ssf")
                    sg = sppool.tile([128, GM], WDT, name="sg", tag="sg")
                    ssp = sppool.tile([128, SM], WDT, name="ssp", tag="ssp")
                    nc.scalar.activation(sgf[:], Vg[:], AF.Sigmoid, bias=bias_th[:], scale=0.5)
                    nc.scalar.activation(ssf[:], Vs[:], AF.Sigmoid, bias=bias_th[:], scale=0.5)
                    nc.vector.tensor_copy(sg[:], sgf[:])
                    nc.vector.tensor_copy(ssp[:], ssf[:])

                    # outgoing gc->small partial + small->small shard
                    ps_sm = smpsum.tile([128, PAY], F32, name="ps_sm", tag="ps_sm")
                    for m in range(SM):
                        for k in range(GM):
                            nc.tensor.matmul(
                                ps_sm[:, m:m + 1],
                                w_out[:, (k * SM + m) * 128:(k * SM + m + 1) * 128],
                                sg[:, k:k + 1],
                                start=(k == 0), stop=(k == GM - 1))
                    for m in range(SSB):
                        for k in range(KT_MC):                 # E rows (mc)
                            nc.tensor.matmul(
                                ps_sm[:, SM + m:SM + m + 1],
                                w_ss[:, (k * SSB + m) * 128:(k * SSB + m + 1) * 128],
                                ssp[:, k:k + 1],
                                start=(k == 0), stop=(k == KT_MC - 1))
                        for k in range(KT_MC, SM):             # I rows (pv, sst)
                            nc.tensor.matmul(
                                ps_sm[:, SM + SSB + m:SM + SSB + m + 1],
                                w_ss[:, (k * SSB + m) * 128:(k * SSB + m + 1) * 128],
                                ssp[:, k:k + 1],
                                start=(k == KT_MC), stop=(k == SM - 1))

                    pay_in = dram.tile([128, PAY], F32, name="pay_in", tag="pay_in")
                    pay_out = dram.tile([NCORES * 128, PAY], F32, addr_space="Shared",
                                        name="pay_out", tag="pay_out")
                    pay_sb = sppool.tile([128, PAY], F32, name="pay_sb", tag="pay_sb")
                    nc.vector.tensor_copy(pay_sb[:], ps_sm[:])
                    nc.sync.dma_start(pay_in[:], pay_sb[:])
                    nc.gpsimd.collective_compute(
                        "AllGather", mybir.AluOpType.bypass, replica_groups=rg,
                        ins=[pay_in.opt()], outs=[pay_out.opt()])

                    # incoming small->gc
                    ps_gc = gcpsum.tile([128, 2 * GM], F32, name="ps_gc", tag="ps_gc")
                    for m in range(GM):
                        for k in range(KT_MC):
                            nc.tensor.matmul(
                                ps_gc[:, m:m + 1],
                                w_ine[:, (k * GM + m) * 128:(k * GM + m + 1) * 128],
                                ssp[:, k:k + 1],
                                start=(k == 0), stop=(k == KT_MC - 1))
                        for k in range(KT_INI):
                            nc.tensor.matmul(
                                ps_gc[:, GM + m:GM + m + 1],
                                w_ini[:, (k * GM + m) * 128:(k * GM + m + 1) * 128],
                                ssp[:, KT_MC + k:KT_MC + k + 1],
                                start=(k == 0), stop=(k == KT_INI - 1))

                    # GC state update
                    _upd(nc, tmppool, Vg, gEg, gIg,
                         ps_gc[:, 0:GM], ps_gc[:, GM:2 * GM], sgf, GM,
                         drive_e=(gEg[:], drive_gc[:, t::t_steps]))
                    nc.sync.dma_start(vg_out_d[t], Vg[:])

                    # gather + merge small increments
                    ag = agpool.tile([128, NCORES, PAY], F32, name="ag", tag="ag")
                    nc.sync.dma_start(
                        ag[:], pay_out[:].rearrange("(i p) j -> p i j", p=128))
                    nc.vector.tensor_copy(incE[:], ag[:, 0, 0:SM])
                    for j in range(1, NCORES):
                        nc.vector.tensor_add(incE[:], incE[:], ag[:, j, 0:SM])
                    for j in range(NCORES - 1):
                        nc.vector.tensor_add(
                            incE[:, SSB * j:SSB * (j + 1)],
                            incE[:, SSB * j:SSB * (j + 1)], ag[:, j, SM:SM + SSB])
                        nc.vector.tensor_copy(
                            incI[:, SSB * j:SSB * (j + 1)], ag[:, j, SM + SSB:PAY])

                    # small-pop state update (replicated on all cores)
                    _upd(nc, tmppool, Vs, gEs, gIs, incE[:], incI[:], ssf, SM,
                         drive_e=(gEs[:, KT_MC:2 * KT_MC], drive_pv[:, t::t_steps]))
    _split_waits(nc, k=1)
    return nc


def _pack(w, kt, mt):
    """[kt*128, mt*128] host matrix -> [128, kt*mt*128] SBUF image with
    block (k, m) at columns [(k*mt+m)*128, (k*mt+m+1)*128)."""
    return np.ascontiguousarray(
        w.reshape(kt, 128, mt, 128).transpose(1, 0, 2, 3).reshape(128, kt * mt * 128)
    )


def make_inputs(core, t_steps, mec_input, W_gc_mc, W_gc_pv, W_gc_sst, W_mc_gc,
                W_mc_pv, W_mc_sst, W_mc_mc, W_mec_gc, W_mec_pv, W_pv_gc,
                W_pv_mc, W_pv_pv, W_pv_sst, W_sst_gc, W_sst_mc, W_sst_pv,
                W_sst_sst):
    sl = slice(G * core, G * (core + 1))
    bf = lambda a: a.astype(np.float16)

    w_ine = np.zeros((KT_MC * 128, GP), np.float32)
    w_ine[:N_MC, :G] = W_mc_gc[:, sl]
    w_ini = np.zeros((KT_INI * 128, GP), np.float32)
    w_ini[:N_PV, :G] = W_pv_gc[:, sl]
    w_ini[KT_PV * 128:KT_PV * 128 + N_SST, :G] = W_sst_gc[:, sl]
    w_out = np.zeros((GP, SMP), np.float32)
    w_out[:G, 0:N_MC] = W_gc_mc[sl]
    w_out[:G, KT_MC * 128:KT_MC * 128 + N_PV] = W_gc_pv[sl]
    w_out[:G, (KT_MC + KT_PV) * 128:(KT_MC + KT_PV) * 128 + N_SST] = W_gc_sst[sl]

    w_ss_full = np.zeros((SMP, SMP), np.float32)
    r_mc, r_pv, r_sst = 0, KT_MC * 128, (KT_MC + KT_PV) * 128
    for r0, wmc, wpv, wsst, n in (
            (r_mc, W_mc_mc, W_mc_pv, W_mc_sst, N_MC),
            (r_pv, W_pv_mc, W_pv_pv, W_pv_sst, N_PV),
            (r_sst, W_sst_mc, W_sst_pv, W_sst_sst, N_SST)):
        w_ss_full[r0:r0 + n, r_mc:r_mc + N_MC] = wmc
        w_ss_full[r0:r0 + n, r_pv:r_pv + N_PV] = wpv
        w_ss_full[r0:r0 + n, r_sst:r_sst + N_SST] = wsst
    c0 = SSB * 128 * core
    w_ss = np.zeros((SMP, SSB * 128), np.float32)
    if c0 < SMP:
        w = min(SSB * 128, SMP - c0)
        w_ss[:, :w] = w_ss_full[:, c0:c0 + w]

    w_mecgc = np.zeros((KT_MEC * 128, GP), np.float32)
    w_mecgc[:N_MEC, :G] = W_mec_gc[:, sl]
    w_mecpv = np.zeros((KT_MEC * 128, KT_PV * 128), np.float32)
    w_mecpv[:N_MEC, :N_PV] = W_mec_pv
    mect = np.zeros((KT_MEC * 128, t_steps), np.float32)
    mect[:N_MEC] = mec_input[:t_steps].T
    mect_sb = np.ascontiguousarray(
        mect.reshape(KT_MEC, 128, t_steps).transpose(1, 0, 2).reshape(128, -1))

    return {
        "w_ine": bf(_pack(w_ine, KT_MC, GM)),
        "w_ini": bf(_pack(w_ini, KT_INI, GM)),
        "w_out": bf(_pack(w_out, GM, SM)),
        "w_ss": bf(_pack(w_ss, SM, SSB)),
        "w_mecgc": bf(_pack(w_mecgc, KT_MEC, GM)),
        "w_mecpv": bf(_pack(w_mecpv, KT_MEC, KT_PV)),
        "mect": bf(mect_sb),
    }


_CACHE = {}


def _get_program(t_steps):
    if t_steps not in _CACHE:
        _CACHE[t_steps] = build_program(t_steps)
    return _CACHE[t_steps]


def run(t_steps, trace=False, **inputs):
    nc = _get_program(t_steps)
    in_maps = [make_inputs(c, t_steps, **inputs) for c in range(NCORES)]
    res = run_bass_kernel_spmd(nc, in_maps, list(range(NCORES)), trace=trace)
    chunks = []
    for c in range(NCORES):
        v = res.results[c]["vg_out"]          # [T, 128, GM]
        chunks.append(v.transpose(0, 2, 1).reshape(t_steps, GP)[:, :G])
    out = np.concatenate(chunks, axis=1).astype(np.float32)
    return out, res


def kernel(**inputs):
    out, _ = run(T, trace=False, **inputs)
    return out


# revision 14
# speedup vs baseline: 2.3541x; 2.3541x over previous
"""Dentate gyrus circuit simulation kernel for 8 Trainium2 NeuronCores.

Strategy
--------
Shard the 20000 GC neurons across 8 cores (2500 each, padded to 2560 =
128 partitions x 20 columns). All weights live SBUF-resident in fp16;
every per-step matvec runs on the TensorEngine with the weight tile as
the stationary operand (lhsT) and the spike vector column as the moving
operand, so results land partition-parallel in PSUM.

The MEC spike train is a known input, so the mec->gc and mec->pv
contributions for all T steps are precomputed with one real matmul at
kernel start, removing W_mec_* from the per-step loop entirely.

Per step, each core:
  - computes local GC spikes + (replicated) small-pop spikes,
  - GC-outgoing partial increments to [mc|pv|sst] targets (sum over its
    2500 GC) plus its 256-column shard of the small->small increments,
  - publishes the [128, 18] fp32 partial via AllGather (8 cores),
  - computes GC-incoming increments from the replicated small spikes,
  - updates GC state locally and small-pop state redundantly
    (identically on every core) from the gathered partials.

Small-population canonical layout (partition-minor, id = p + 128*f):
  mc at [0,600) cols 0:5, pv at [640,1240) cols 5:10,
  sst at [1280,1680) cols 10:14; 1792 = 14 cols total, pads have zero
  weights. GC per-core canonical id c = p + 128*f, cols 0:20.
"""
import sys
sys.path.insert(0, '/opt/trn_rl_repo')
import numpy as np

import concourse.bass as bass
import concourse.mybir as mybir
import concourse.tile as tile
from concourse.bass_utils import run_bass_kernel_spmd

F32 = mybir.dt.float32
WDT = mybir.dt.float16
AF = mybir.ActivationFunctionType

N_GC, N_MC, N_PV, N_SST, N_MEC = 20000, 600, 600, 400, 1200
T = 100
DT = 0.1
TAU_AMPA, TAU_GABA, TAU_M = 2.0, 10.0, 20.0
V_REST, V_TH, V_RESET = -70.0, -50.0, -70.0
D_AMPA = float(np.exp(-DT / TAU_AMPA))
D_GABA = float(np.exp(-DT / TAU_GABA))

NCORES = 8
G = N_GC // NCORES          # 2500 gc per core
GM = 20                     # gc column count (2560 = 128*20)
GP = 128 * GM
SM = 14                     # small-pop column count (1792 = 128*14)
SMP = 128 * SM
SSB = 2                     # small->small target cols per core (256 ids)
# k-tile counts
KT_MC, KT_PV, KT_SST = 5, 5, 4
KT_INI = KT_PV + KT_SST     # incoming inhibitory k-tiles
KT_MEC = 10                 # mec padded to 1280
PAY = SM + 2 * SSB          # collective payload cols: po(14)|ssE(2)|ssI(2)


class SafeTileContext(tile.TileContext):
    """TileContext whose exit drain carries no sem waits (this walrus
    rejects sync waits on the Drain encoding); waits go on single-wait
    NOPs instead."""

    def _drain_and_barrier(self, tick_clock, wait_clock):
        probe = self.nc.sync.nop(nofuse=True, hint="exit_wait_probe")
        wait_clock.add_sem_waits(
            probe.ins, tile.ScopedClock({None: tick_clock.global_clock})
        )
        waits = list(probe.ins.sync_info.on_wait) if probe.ins.sync_info else []
        if probe.ins.sync_info is not None:
            probe.ins.sync_info = None
        for w in waits:
            n = self.nc.sync.nop(nofuse=True, hint="exit_wait")
            n.ins.sync_info = mybir.SyncInfo(on_wait=[w], on_update=[])
        self.nc.sync.drain()
        self.nc.all_engine_barrier()
        assert self.sems is not None
        popped = self.nc._tile_sem_poison_stack.pop()
        assert popped is self._sem_poison
        self.nc.clear_and_free_semaphores(list(self.sems.allocated().values()))
        self.nc.all_engine_barrier()


def _split_waits(nc, k=1):
    """Hoist sem waits beyond ``k`` per instruction onto fresh same-engine
    NOPs placed immediately before — this walrus build rejects multi-wait
    sync commands on several instruction encodings."""
    ctr = 0
    for bb in nc.main_func.blocks:
        insts = list(bb.instructions)
        out = []
        changed = False
        for inst in insts:
            si = inst.sync_info
            if si is not None and si.on_wait and len(si.on_wait) > k:
                waits = list(si.on_wait)
                extra, keep = waits[:-k], waits[-k:]
                for i in range(0, len(extra), k):
                    nop = mybir.InstNoOp(name=f"waitnop-{ctr}", ins=[], outs=[])
                    ctr += 1
                    nop.engine = inst.engine
                    nop.sync_info = mybir.SyncInfo(
                        on_wait=extra[i:i + k], on_update=[])
                    nc.register_instruction(nop)
                    out.append(nop)
                inst.sync_info = mybir.SyncInfo(
                    on_wait=keep, on_update=list(si.on_update))
                changed = True
            out.append(inst)
        if changed:
            bb.instructions[:] = out
    return ctr


def _upd(nc, pool, V, gE, gI, psE, psI, s_f32, ncols, drive_e=None):
    """One Euler step + soft reset for a population laid out [128, ncols].

    gE/gI decay + increment, I = gE*(0-V) + gI*(-70-V),
    Vn = V + DT*((V_REST-V)/TAU_M + I), V' = Vn - (Vn - V_RESET)*s.
    """
    a = 1.0 - DT / TAU_M
    b = (DT / TAU_M) * V_REST
    t1 = pool.tile([128, ncols], F32, name=f"t1_{ncols}", tag=f"t1_{ncols}")
    t2 = pool.tile([128, ncols], F32, name=f"t2_{ncols}", tag=f"t2_{ncols}")
    t3 = pool.tile([128, ncols], F32, name=f"t3_{ncols}", tag=f"t3_{ncols}")
    nc.vector.tensor_scalar_mul(gE[:], gE[:], D_AMPA)
    nc.vector.tensor_add(gE[:], gE[:], psE)
    if drive_e is not None:
        dst, src = drive_e
        nc.vector.tensor_add(dst, dst, src)
    nc.vector.tensor_scalar_mul(gI[:], gI[:], D_GABA)
    nc.vector.tensor_add(gI[:], gI[:], psI)
    nc.vector.tensor_add(t1[:], gE[:], gI[:])          # w = gE + gI
    nc.vector.tensor_mul(t1[:], t1[:], V[:])           # u = w * V
    nc.vector.tensor_scalar(t2[:], V[:], a, b, mybir.AluOpType.mult,
                            mybir.AluOpType.add)       # Vn = a*V + b
    nc.vector.tensor_scalar_mul(t1[:], t1[:], -DT)
    nc.vector.tensor_add(t2[:], t2[:], t1[:])          # Vn -= DT*u
    nc.vector.tensor_scalar_mul(t3[:], gI[:], -70.0 * DT)
    nc.vector.tensor_add(t2[:], t2[:], t3[:])          # Vn -= 70*DT*gI
    # soft reset: V' = Vn - (Vn - V_RESET)*s
    nc.vector.tensor_scalar_add(t3[:], t2[:], -V_RESET)
    nc.vector.tensor_mul(t3[:], t3[:], s_f32[:])
    nc.vector.tensor_sub(V[:], t2[:], t3[:])


def build_program(t_steps):
    nc = bass.Bass(num_devices=NCORES)

    w_ine_d = nc.dram_tensor("w_ine", [128, KT_MC * GP], WDT, kind="ExternalInput")
    w_ini_d = nc.dram_tensor("w_ini", [128, KT_INI * GP], WDT, kind="ExternalInput")
    w_out_d = nc.dram_tensor("w_out", [128, GM * SMP], WDT, kind="ExternalInput")
    w_ss_d = nc.dram_tensor("w_ss", [128, SM * SSB * 128], WDT, kind="ExternalInput")
    w_mecgc_d = nc.dram_tensor("w_mecgc", [128, KT_MEC * GP], WDT, kind="ExternalInput")
    w_mecpv_d = nc.dram_tensor("w_mecpv", [128, KT_MEC * KT_PV * 128], WDT, kind="ExternalInput")
    mect_d = nc.dram_tensor("mect", [128, KT_MEC * t_steps], WDT, kind="ExternalInput")
    vg_out_d = nc.dram_tensor("vg_out", [t_steps, 128, GM], F32, kind="ExternalOutput")

    rg = [list(range(NCORES))]

    with SafeTileContext(nc) as tc:
        with tc.tile_pool(name="persist", bufs=1) as wpool:
            drive_gc = wpool.tile([128, GM * t_steps], F32)   # col m*T + t
            drive_pv = wpool.tile([128, KT_PV * t_steps], F32)

            Vg = wpool.tile([128, GM], F32)
            gEg = wpool.tile([128, GM], F32)
            gIg = wpool.tile([128, GM], F32)
            Vs = wpool.tile([128, SM], F32)
            gEs = wpool.tile([128, SM], F32)
            gIs = wpool.tile([128, SM], F32)
            incE = wpool.tile([128, SM], F32)
            incI = wpool.tile([128, SM], F32)
            bias_th = wpool.tile([128, 1], F32)
            nc.vector.memset(bias_th[:], -V_TH / 2.0)
            nc.vector.memset(Vg[:], V_REST)
            nc.vector.memset(gEg[:], 0.0)
            nc.vector.memset(gIg[:], 0.0)
            nc.vector.memset(Vs[:], V_REST)
            nc.vector.memset(gEs[:], 0.0)
            nc.vector.memset(gIs[:], 0.0)

            # ---- Phase 1: precompute mec drives, then free mec weights ----
            with (
                tc.tile_pool(name="mecpool", bufs=1) as mecpool,
                tc.tile_pool(name="pcpsum", bufs=4, space="PSUM") as pcpsum,
            ):
                wm_gc = mecpool.tile([128, KT_MEC * GP], WDT)
                wm_pv = mecpool.tile([128, KT_MEC * KT_PV * 128], WDT)
                smect = mecpool.tile([128, KT_MEC * t_steps], WDT)
                nc.sync.dma_start(wm_gc[:], w_mecgc_d[:])
                nc.sync.dma_start(wm_pv[:], w_mecpv_d[:])
                nc.sync.dma_start(smect[:], mect_d[:])
                for m in range(GM):
                    ps = pcpsum.tile([128, t_steps], F32, name="pcps", tag="pcps")
                    for k in range(KT_MEC):
                        nc.tensor.matmul(
                            ps[:], wm_gc[:, (k * GM + m) * 128:(k * GM + m + 1) * 128],
                            smect[:, k * t_steps:(k + 1) * t_steps],
                            start=(k == 0), stop=(k == KT_MEC - 1))
                    nc.vector.tensor_copy(drive_gc[:, m * t_steps:(m + 1) * t_steps], ps[:])
                for m in range(KT_PV):
                    ps = pcpsum.tile([128, t_steps], F32, name="pcps", tag="pcps")
                    for k in range(KT_MEC):
                        nc.tensor.matmul(
                            ps[:], wm_pv[:, (k * KT_PV + m) * 128:(k * KT_PV + m + 1) * 128],
                            smect[:, k * t_steps:(k + 1) * t_steps],
                            start=(k == 0), stop=(k == KT_MEC - 1))
                    nc.vector.tensor_copy(drive_pv[:, m * t_steps:(m + 1) * t_steps], ps[:])

            # ---- Phase 2: resident weights (reuse freed mec space) ----
            with (
                tc.tile_pool(name="mainw", bufs=1) as mainw,
                tc.tile_pool(name="sppool", bufs=3) as sppool,
                tc.tile_pool(name="tmppool", bufs=2) as tmppool,
                tc.tile_pool(name="agpool", bufs=2) as agpool,
                tc.tile_pool(name="gcpsum", bufs=2, space="PSUM") as gcpsum,
                tc.tile_pool(name="smpsum", bufs=2, space="PSUM") as smpsum,
                tc.tile_pool(name="dram", bufs=2, space="DRAM") as dram,
            ):
                w_ine = mainw.tile([128, KT_MC * GP], WDT)
                w_ini = mainw.tile([128, KT_INI * GP], WDT)
                w_out = mainw.tile([128, GM * SMP], WDT)
                w_ss = mainw.tile([128, SM * SSB * 128], WDT)
                nc.sync.dma_start(w_ine[:], w_ine_d[:])
                nc.sync.dma_start(w_ini[:], w_ini_d[:])
                nc.sync.dma_start(w_out[:], w_out_d[:])
                nc.sync.dma_start(w_ss[:], w_ss_d[:])

                for t in range(t_steps):
                    # spikes: s = sigmoid((V - V_TH)/2) = sigmoid(0.5*V + 10)
                    sgf = sppool.tile([128, GM], F32, name="sgf", tag="sgf")
                    ssf = sppool.tile([128, SM], F32, name="ssf", tag="ssf")
                    sg = sppool.tile([128, GM], WDT, name="sg", tag="sg")
                    ssp = sppool.tile([128, SM], WDT, name="ssp", tag="ssp")
                    nc.scalar.activation(sgf[:], Vg[:], AF.Sigmoid, bias=bias_th[:], scale=0.5)
                    nc.scalar.activation(ssf[:], Vs[:], AF.Sigmoid, bias=bias_th[:], scale=0.5)
                    nc.vector.tensor_copy(sg[:], sgf[:])
                    nc.vector.tensor_copy(ssp[:], ssf[:])

                    # outgoing gc->small partial + small->small shard
                    ps_sm = smpsum.tile([128, PAY], F32, name="ps_sm", tag="ps_sm")
                    for m in range(SM):
                        for k in range(GM):
                            nc.tensor.matmul(
                                ps_sm[:, m:m + 1],
                                w_out[:, (k * SM + m) * 128:(k * SM + m + 1) * 128],
                                sg[:, k:k + 1],
                                start=(k == 0), stop=(k == GM - 1))
                    for m in range(SSB):
                        for k in range(KT_MC):                 # E rows (mc)
                            nc.tensor.matmul(
                                ps_sm[:, SM + m:SM + m + 1],
                                w_ss[:, (k * SSB + m) * 128:(k * SSB + m + 1) * 128],
                                ssp[:, k:k + 1],
                                start=(k == 0), stop=(k == KT_MC - 1))
                        for k in range(KT_MC, SM):             # I rows (pv, sst)
                            nc.tensor.matmul(
                                ps_sm[:, SM + SSB + m:SM + SSB + m + 1],
                                w_ss[:, (k * SSB + m) * 128:(k * SSB + m + 1) * 128],
                                ssp[:, k:k + 1],
                                start=(k == KT_MC), stop=(k == SM - 1))

                    pay_in = dram.tile([128, PAY], F32, name="pay_in", tag="pay_in")
                    pay_out = dram.tile([NCORES * 128, PAY], F32, addr_space="Shared",
                                        name="pay_out", tag="pay_out")
                    pay_sb = sppool.tile([128, PAY], F32, name="pay_sb", tag="pay_sb")
                    nc.vector.tensor_copy(pay_sb[:], ps_sm[:])
                    nc.sync.dma_start(pay_in[:], pay_sb[:])
                    nc.gpsimd.collective_compute(
                        "AllGather", mybir.AluOpType.bypass, replica_groups=rg,
                        ins=[pay_in.opt()], outs=[pay_out.opt()])

                    # incoming small->gc
                    ps_gc = gcpsum.tile([128, 2 * GM], F32, name="ps_gc", tag="ps_gc")
                    for m in range(GM):
                        for k in range(KT_MC):
                            nc.tensor.matmul(
                                ps_gc[:, m:m + 1],
                                w_ine[:, (k * GM + m) * 128:(k * GM + m + 1) * 128],
                                ssp[:, k:k + 1],
                                start=(k == 0), stop=(k == KT_MC - 1))
                        for k in range(KT_INI):
                            nc.tensor.matmul(
                                ps_gc[:, GM + m:GM + m + 1],
                                w_ini[:, (k * GM + m) * 128:(k * GM + m + 1) * 128],
                                ssp[:, KT_MC + k:KT_MC + k + 1],
                                start=(k == 0), stop=(k == KT_INI - 1))

                    # GC state update
                    _upd(nc, tmppool, Vg, gEg, gIg,
                         ps_gc[:, 0:GM], ps_gc[:, GM:2 * GM], sgf, GM,
                         drive_e=(gEg[:], drive_gc[:, t::t_steps]))
                    nc.sync.dma_start(vg_out_d[t], Vg[:])

                    # gather + merge small increments
                    ag = agpool.tile([128, NCORES, PAY], F32, name="ag", tag="ag")
                    nc.sync.dma_start(
                        ag[:], pay_out[:].rearrange("(i p) j -> p i j", p=128))
                    nc.vector.tensor_copy(incE[:], ag[:, 0, 0:SM])
                    for j in range(1, NCORES):
                        nc.vector.tensor_add(incE[:], incE[:], ag[:, j, 0:SM])
                    for j in range(NCORES - 1):
                        nc.vector.tensor_add(
                            incE[:, SSB * j:SSB * (j + 1)],
                            incE[:, SSB * j:SSB * (j + 1)], ag[:, j, SM:SM + SSB])
                        nc.vector.tensor_copy(
                            incI[:, SSB * j:SSB * (j + 1)], ag[:, j, SM + SSB:PAY])

                    # small-pop state update (replicated on all cores)
                    _upd(nc, tmppool, Vs, gEs, gIs, incE[:], incI[:], ssf, SM,
                         drive_e=(gEs[:, KT_MC:2 * KT_MC], drive_pv[:, t::t_steps]))
    _split_waits(nc, k=1)
    return nc


def _pack(w, kt, mt):
    """[kt*128, mt*128] host matrix -> [128, kt*mt*128] SBUF image with
    block (k, m) at columns [(k*mt+m)*128, (k*mt+m+1)*128)."""
    return np.ascontiguousarray(
        w.reshape(kt, 128, mt, 128).transpose(1, 0, 2, 3).reshape(128, kt * mt * 128)
    )


def make_inputs(core, t_steps, mec_input, W_gc_mc, W_gc_pv, W_gc_sst, W_mc_gc,
                W_mc_pv, W_mc_sst, W_mc_mc, W_mec_gc, W_mec_pv, W_pv_gc,
                W_pv_mc, W_pv_pv, W_pv_sst, W_sst_gc, W_sst_mc, W_sst_pv,
                W_sst_sst):
    sl = slice(G * core, G * (core + 1))
    bf = lambda a: a.astype(np.float16)

    w_ine = np.zeros((KT_MC * 128, GP), np.float32)
    w_ine[:N_MC, :G] = W_mc_gc[:, sl]
    w_ini = np.zeros((KT_INI * 128, GP), np.float32)
    w_ini[:N_PV, :G] = W_pv_gc[:, sl]
    w_ini[KT_PV * 128:KT_PV * 128 + N_SST, :G] = W_sst_gc[:, sl]
    w_out = np.zeros((GP, SMP), np.float32)
    w_out[:G, 0:N_MC] = W_gc_mc[sl]
    w_out[:G, KT_MC * 128:KT_MC * 128 + N_PV] = W_gc_pv[sl]
    w_out[:G, (KT_MC + KT_PV) * 128:(KT_MC + KT_PV) * 128 + N_SST] = W_gc_sst[sl]

    w_ss_full = np.zeros((SMP, SMP), np.float32)
    r_mc, r_pv, r_sst = 0, KT_MC * 128, (KT_MC + KT_PV) * 128
    for r0, wmc, wpv, wsst, n in (
            (r_mc, W_mc_mc, W_mc_pv, W_mc_sst, N_MC),
            (r_pv, W_pv_mc, W_pv_pv, W_pv_sst, N_PV),
            (r_sst, W_sst_mc, W_sst_pv, W_sst_sst, N_SST)):
        w_ss_full[r0:r0 + n, r_mc:r_mc + N_MC] = wmc
        w_ss_full[r0:r0 + n, r_pv:r_pv + N_PV] = wpv
        w_ss_full[r0:r0 + n, r_sst:r_sst + N_SST] = wsst
    c0 = SSB * 128 * core
    w_ss = np.zeros((SMP, SSB * 128), np.float32)
    if c0 < SMP:
        w = min(SSB * 128, SMP - c0)
        w_ss[:, :w] = w_ss_full[:, c0:c0 + w]

    w_mecgc = np.zeros((KT_MEC * 128, GP), np.float32)
    w_mecgc[:N_MEC, :G] = W_mec_gc[:, sl]
    w_mecpv = np.zeros((KT_MEC * 128, KT_PV * 128), np.float32)
    w_mecpv[:N_MEC, :N_PV] = W_mec_pv
    mect = np.zeros((KT_MEC * 128, t_steps), np.float32)
    mect[:N_MEC] = mec_input[:t_steps].T
    mect_sb = np.ascontiguousarray(
        mect.reshape(KT_MEC, 128, t_steps).transpose(1, 0, 2).reshape(128, -1))

    return {
        "w_ine": bf(_pack(w_ine, KT_MC, GM)),
        "w_ini": bf(_pack(w_ini, KT_INI, GM)),
        "w_out": bf(_pack(w_out, GM, SM)),
        "w_ss": bf(_pack(w_ss, SM, SSB)),
        "w_mecgc": bf(_pack(w_mecgc, KT_MEC, GM)),
        "w_mecpv": bf(_pack(w_mecpv, KT_MEC, KT_PV)),
        "mect": bf(mect_sb),
    }


_CACHE = {}


def _get_program(t_steps):
    if t_steps not in _CACHE:
        _CACHE[t_steps] = build_program(t_steps)
    return _CACHE[t_steps]


def run(t_steps, trace=False, **inputs):
    nc = _get_program(t_steps)
    in_maps = [make_inputs(c, t_steps, **inputs) for c in range(NCORES)]
    res = run_bass_kernel_spmd(nc, in_maps, list(range(NCORES)), trace=trace)
    chunks = []
    for c in range(NCORES):
        v = res.results[c]["vg_out"]          # [T, 128, GM]
        chunks.append(v.transpose(0, 2, 1).reshape(t_steps, GP)[:, :G])
    out = np.concatenate(chunks, axis=1).astype(np.float32)
    return out, res


def kernel(**inputs):
    out, _ = run(T, trace=False, **inputs)
    return out
